# revision 35
# baseline (speedup 1.0000x reference)
"""Trainium2 Bass kernel for nn_DWAttEncoder (depth-wise attention encoder).

Strategy (8 NeuronCores):
  The single-query attention over depth is computed on host (fp32; ~1.5% of
  FLOPs, softmax-logit precision critical) exactly as before.  With the
  resulting per-row attention weights known before launch, the per-layer
  value-MLP work is *sparsified*: for each batch row only the layers whose
  attention mass matters (residual mass < 1e-4) are computed.  The selected
  (row, layer) pairs are gathered into fixed-shape per-layer row batches
  ("slots") that the device kernel processes:

      per slot:  h = gelu(xg @ W1[l]);  h = LN(h)
                 v = LN(h @ W2[l]) * attn        (attn folded into LN2 affine)

  Slots: 8 cores x NSLOT slots x RP rows, SPMD (same program, per-core data).
  Layers whose selected-row count doesn't fit the slot budget are computed
  on host in fp32 (small tail).  Host scatter-adds all contributions into
  z_L.  b1/b2 zeros and ln*_g/ln*_b ones/zeros per the problem spec are
  verified at runtime, with a full-precision host fallback if ever not.

  Weight stacks are staged n-chunk-major so the first matmul chain only
  waits for its own 512-wide slice of W1, not the full tensor.
"""

import numpy as np
import ml_dtypes

import concourse.bacc as bacc
import concourse.tile as tile
from concourse import mybir
from concourse.bass_utils import run_bass_kernel_spmd
from concourse.masks import make_identity

BF16_NP = ml_dtypes.bfloat16
BF16 = mybir.dt.bfloat16
F16 = mybir.dt.float16
F32 = mybir.dt.float32
AF = mybir.ActivationFunctionType
OP = mybir.AluOpType

L, D, DB = 33, 2048, 1024
B = 2048
NCORES = 8
RPS = (384, 256)      # rows per slot (hybrid slot sizes, per core)
NSLOT = len(RPS)
RPMAX = max(RPS)
TOTAL_SLOTS = NCORES * NSLOT
KD = D // 128         # 16 k-tiles for mm1 contraction
KB = DB // 128        # 8 k-tiles for mm2 contraction
ND1 = DB // 512       # 2 psum chunks for h
ND2 = D // 512        # 4 psum chunks for v
EPS = 1e-5
EPS_MASS = 1e-4       # max attention mass dropped per row
NWARM = 48            # PE warm-up transposes (HAM un-throttle during DMA head)

_cached_nc = None
_last_results = None


def _build():
    global _cached_nc
    if _cached_nc is not None:
        return _cached_nc

    nc = bacc.Bacc("TRN2", target_bir_lowering=False, debug=False,
                   num_devices=NCORES)
    MBMAX = RPMAX // 128
    xt = nc.dram_tensor("xt", [NSLOT, KD, 128, RPMAX], BF16,
                        kind="ExternalInput")
    w1 = nc.dram_tensor("w1", [NSLOT, ND1, KD, 128, 512], BF16,
                        kind="ExternalInput")
    w2 = nc.dram_tensor("w2", [NSLOT, ND2, KB, 128, 512], BF16,
                        kind="ExternalInput")
    attn = nc.dram_tensor("attn", [NSLOT, MBMAX, 128], F32,
                          kind="ExternalInput")
    out = nc.dram_tensor("out", [NSLOT, MBMAX, 128, D], F16,
                         kind="ExternalOutput")
    warm = nc.dram_tensor("warm", [128, 128], BF16, kind="ExternalOutput")

    with tile.TileContext(nc) as tc:
        with (
            tc.tile_pool(name="const", bufs=1) as cpool,
            tc.tile_pool(name="w1p", bufs=2) as w1p,
            tc.tile_pool(name="w2p", bufs=2) as w2p,
            tc.tile_pool(name="xtp", bufs=2) as xtp,
            tc.tile_pool(name="hp", bufs=3) as hp,
            tc.tile_pool(name="htp", bufs=3) as htp,
            tc.tile_pool(name="outp", bufs=4) as outp,
            tc.tile_pool(name="stats", bufs=8) as stats,
            tc.tile_pool(name="ph", bufs=2, space="PSUM") as php,
            tc.tile_pool(name="pt", bufs=2, space="PSUM") as ptp,
            tc.tile_pool(name="pv", bufs=4, space="PSUM") as pvp,
        ):
            def emit_loads(s, kf):
                """Issue the DMA loads for slot ``s``; weight chunks are
                ordered to match matmul consumption (w1 n-chunk 0 first,
                then n-chunk 1, then w2 by n-chunk).  ``kf`` = k-tiles per
                DMA chunk."""
                rp = RPS[s]
                xt_sb = xtp.tile([128, KD, rp], BF16, tag="xt")
                if s == 0:
                    # m-tile-major: the first mm1 chain only needs xt cols
                    # 0:128, so those land first
                    for mi in range(rp // 128):
                        bsl = slice(mi * 128, (mi + 1) * 128)
                        for c in range(0, KD, 8):
                            nc.sync.dma_start(
                                out=xt_sb[:, c:c + 8, bsl],
                                in_=xt[s, c:c + 8, :, bsl].rearrange(
                                    "k p b -> p k b"))
                else:
                    for c in range(0, KD, 4):
                        nc.sync.dma_start(
                            out=xt_sb[:, c:c + 4, :],
                            in_=xt[s, c:c + 4, :, :rp].rearrange(
                                "k p b -> p k b"))
                w1_sb = w1p.tile([128, ND1, KD, 512], BF16, tag="w1")
                if s == 0:
                    # n-major: the first (n-outer) chain of slot 0 consumes
                    # w1 n-chunk 0 for all k before touching n-chunk 1
                    for n in range(ND1):
                        for c in range(0, KD, kf):
                            nc.sync.dma_start(
                                out=w1_sb[:, n, c:c + kf, :],
                                in_=w1[s, n, c:c + kf].rearrange(
                                    "k p e -> p k e"))
                else:
                    for c in range(0, KD, kf):
                        for n in range(ND1):
                            nc.sync.dma_start(
                                out=w1_sb[:, n, c:c + kf, :],
                                in_=w1[s, n, c:c + kf].rearrange(
                                    "k p e -> p k e"))
                w2_sb = w2p.tile([128, ND2, KB, 512], BF16, tag="w2")
                for c in range(0, KB, kf):
                    for n in range(ND2):
                        nc.sync.dma_start(
                            out=w2_sb[:, n, c:c + kf, :],
                            in_=w2[s, n, c:c + kf].rearrange("k p e -> p k e"))
                return xt_sb, w1_sb, w2_sb

            # slot-0 inputs first, finely split, so the first matmul can
            # start as early as possible
            pending = emit_loads(0, 2)

            ident = cpool.tile([128, 128], BF16)
            make_identity(nc, ident)
            eps_t = cpool.tile([128, 1], F32)
            nc.vector.memset(eps_t, EPS)
            attn_sb = cpool.tile([128, NSLOT, MBMAX], F32)
            nc.sync.dma_start(out=attn_sb,
                              in_=attn[:].rearrange("s m p -> p s m"))

            # PE warm-up: a serialized transpose chain keeps the PE busy
            # while the first weight DMAs stream in, so the HAM clock gate
            # un-throttles (1.2 -> 2.4 GHz) before real matmuls start.
            wt = ptp.tile([128, 128], BF16, tag="pt")
            for _ in range(NWARM):
                nc.tensor.transpose(wt, ident, ident)
            w_sb = cpool.tile([128, 128], BF16)
            nc.vector.tensor_copy(out=w_sb, in_=wt)
            nc.sync.dma_start(out=warm[:], in_=w_sb)

            for s in range(NSLOT):
                MB2 = RPS[s] // 128
                xt_sb, w1_sb, w2_sb = pending

                # ---- phase A per m-tile: mm1 + gelu + LN1 ----
                h_ln = []
                for m in range(MB2):
                    msl = slice(m * 128, (m + 1) * 128)
                    phs = [php.tile([128, 512], F32, tag="ph", name=f"ph{n}")
                           for n in range(ND1)]
                    if s == 0 and m == 0:
                        # n-outer on the very first tile: chain n0 only waits
                        # for w1 n-chunk 0 + xt cols 0:128 (~2.6 MB), so real
                        # matmuls start ~5 us earlier
                        for n in range(ND1):
                            for k in range(KD):
                                nc.tensor.matmul(
                                    phs[n], lhsT=xt_sb[:, k, msl],
                                    rhs=w1_sb[:, n, k, :],
                                    start=(k == 0), stop=(k == KD - 1))
                    else:
                        # k-outer so the stationary operand (xt k-tile) is
                        # reused by ND1 consecutive matmuls -> fewer exposed
                        # LDWEIGHTS
                        for k in range(KD):
                            for n in range(ND1):
                                nc.tensor.matmul(
                                    phs[n], lhsT=xt_sb[:, k, msl],
                                    rhs=w1_sb[:, n, k, :],
                                    start=(k == 0), stop=(k == KD - 1))
                    hg = hp.tile([128, DB], BF16, tag="hg")
                    for n in range(ND1):
                        nc.scalar.activation(
                            out=hg[:, n * 512:(n + 1) * 512], in_=phs[n],
                            func=AF.Gelu)
                    st1 = stats.tile([128, ND1, 6], F32, tag="st1")
                    for n in range(ND1):
                        nc.vector.bn_stats(
                            out=st1[:, n, :], in_=hg[:, n * 512:(n + 1) * 512])
                    mv1 = stats.tile([128, 2], F32, tag="mv1")
                    nc.vector.bn_aggr(out=mv1, in_=st1)
                    rs1 = stats.tile([128, 1], F32, tag="rs1")
                    nc.scalar.activation(out=rs1, in_=mv1[:, 1:2], func=AF.Sqrt,
                                         bias=eps_t)
                    nc.vector.reciprocal(out=rs1, in_=rs1)
                    c1 = stats.tile([128, 1], F32, tag="c1")
                    nc.vector.tensor_scalar(out=c1, in0=mv1[:, 0:1], scalar1=rs1,
                                            scalar2=-1.0, op0=OP.mult,
                                            op1=OP.mult)
                    hl = hp.tile([128, DB], BF16, tag="hl")
                    for n in range(ND1):
                        nsl = slice(n * 512, (n + 1) * 512)
                        nc.scalar.activation(out=hl[:, nsl], in_=hg[:, nsl],
                                             func=AF.Identity, bias=c1,
                                             scale=rs1)
                    h_ln.append(hl)

                # ---- phase B per m-tile: transpose + mm2 + LN2*attn ----
                for m in range(MB2):
                    # queue the next slot's loads on the sync engine after
                    # the first out-DMAs of this slot, so they neither fight
                    # the head loads nor wait behind every output transfer
                    if m == 1 and s + 1 < NSLOT:
                        pending = emit_loads(s + 1, 4)
                    hl = h_ln[m]
                    ht = htp.tile([128, KB, 128], BF16, tag="ht")
                    for j2 in range(KB // 2):
                        pt = ptp.tile([128, 256], BF16, tag="pt")
                        for jj in range(2):
                            j = j2 * 2 + jj
                            nc.tensor.transpose(
                                pt[:, jj * 128:(jj + 1) * 128],
                                hl[:, j * 128:(j + 1) * 128], ident)
                        nc.vector.tensor_copy(
                            out=ht[:, j2 * 2:j2 * 2 + 2, :],
                            in_=pt[:].rearrange("p (j b) -> p j b", j=2))

                    st2 = stats.tile([128, ND2, 6], F32, tag="st2")
                    pvs = [pvp.tile([128, 512], F32, tag="pv", name=f"pv{n}")
                           for n in range(ND2)]
                    last_tile = (s == NSLOT - 1 and m == MB2 - 1)
                    if last_tile:
                        # n-outer: each chunk's stats start right after its
                        # chain, shortening the un-overlapped kernel tail
                        for n in range(ND2):
                            for k in range(KB):
                                nc.tensor.matmul(
                                    pvs[n], lhsT=ht[:, k, :],
                                    rhs=w2_sb[:, n, k, :],
                                    start=(k == 0), stop=(k == KB - 1))
                            nc.vector.bn_stats(out=st2[:, n, :], in_=pvs[n])
                    else:
                        for k in range(KB):
                            for n in range(ND2):
                                nc.tensor.matmul(
                                    pvs[n], lhsT=ht[:, k, :],
                                    rhs=w2_sb[:, n, k, :],
                                    start=(k == 0), stop=(k == KB - 1))
                        for n in range(ND2):
                            nc.vector.bn_stats(out=st2[:, n, :], in_=pvs[n])
                    mv2 = stats.tile([128, 2], F32, tag="mv2")
                    nc.vector.bn_aggr(out=mv2, in_=st2)
                    rs2 = stats.tile([128, 1], F32, tag="rs2")
                    nc.scalar.activation(out=rs2, in_=mv2[:, 1:2], func=AF.Sqrt,
                                         bias=eps_t)
                    nc.vector.reciprocal(out=rs2, in_=rs2)
                    a2 = stats.tile([128, 1], F32, tag="a2")
                    nc.vector.tensor_mul(out=a2, in0=rs2,
                                         in1=attn_sb[:, s, m:m + 1])
                    c2 = stats.tile([128, 1], F32, tag="c2")
                    nc.vector.tensor_scalar(out=c2, in0=mv2[:, 0:1], scalar1=a2,
                                            scalar2=-1.0, op0=OP.mult,
                                            op1=OP.mult)
                    o_sb = outp.tile([128, D], F16, tag="o")
                    for n in range(ND2):
                        nsl = slice(n * 512, (n + 1) * 512)
                        if last_tile and n % 2:
                            # split the apply across DVE and ACT so the tail
                            # isn't serialized on one engine
                            nc.vector.tensor_scalar(out=o_sb[:, nsl],
                                                    in0=pvs[n], scalar1=a2,
                                                    scalar2=c2, op0=OP.mult,
                                                    op1=OP.add)
                        else:
                            nc.scalar.activation(out=o_sb[:, nsl], in_=pvs[n],
                                                 func=AF.Identity, bias=c2,
                                                 scale=a2)
                        nc.sync.dma_start(out=out[s, m, :, nsl],
                                          in_=o_sb[:, nsl])

    nc.compile()
    _cached_nc = nc
    return nc


# ---------------- host-side math (fp32) ----------------

def _gelu(x):
    from scipy.special import erf
    return 0.5 * x * (1.0 + erf(x / np.sqrt(2.0, dtype=np.float32)))


def _ln(x, g, b):
    mu = x.mean(-1, keepdims=True, dtype=np.float32)
    var = np.square(x - mu).mean(-1, keepdims=True, dtype=np.float32)
    return (x - mu) / np.sqrt(var + EPS) * g + b


def _elu(x):
    return np.where(x > 0, x, np.expm1(np.minimum(x, 0.0)))


def _host_query_attn(zL, pos_emb, Wk, Wq1, bq1, lnq_g, lnq_b, Wq2, bq2):
    keys = pos_emb @ Wk                                   # [L, D]
    hq = _gelu(zL @ Wq1 + bq1)
    hq = _ln(hq, lnq_g, lnq_b)
    q_tr = hq @ Wq2 + bq2
    query = 1.0 + _elu(zL + q_tr)                         # [B, D]
    s = query @ keys.T                                    # [B, L]
    s -= s.max(-1, keepdims=True)
    e = np.exp(s)
    return e / e.sum(-1, keepdims=True)


def _host_reference(x, pos_emb, Wk, W1, b1, ln1_g, ln1_b, W2, b2, ln2_g,
                    ln2_b, Wq1, bq1, lnq_g, lnq_b, Wq2, bq2):
    """Full-precision fallback (only used if the affine params are ever
    non-trivial, which the problem spec's fills make impossible)."""
    zL = x[:, -1, :]
    attn = _host_query_attn(zL, pos_emb, Wk, Wq1, bq1, lnq_g, lnq_b, Wq2, bq2)
    acc = np.zeros_like(zL)
    for l in range(L):
        h = _gelu(x[:, l, :] @ W1[l] + b1[l])
        h = _ln(h, ln1_g[l], ln1_b[l])
        v = h @ W2[l] + b2[l]
        v = _ln(v, ln2_g[l], ln2_b[l])
        acc += attn[:, l:l + 1] * v
    return zL + acc


def _select_pairs(attn):
    """Per-row adaptive selection: keep the smallest set of layers whose
    residual attention mass is < EPS_MASS.  Returns keep mask [B, L]."""
    srt = np.sort(attn, axis=-1)[:, ::-1]
    cum = np.cumsum(srt, axis=-1)
    need = (cum < 1.0 - EPS_MASS).sum(-1) + 1
    need = np.minimum(need, attn.shape[1])
    thresh = srt[np.arange(attn.shape[0]), need - 1]
    return attn >= thresh[:, None]


def kernel(x, pos_emb, Wk, W1, b1, ln1_g, ln1_b, W2, b2, ln2_g, ln2_b,
           Wq1, bq1, lnq_g, lnq_b, Wq2, bq2):
    global _last_results
    f32 = np.float32
    x = np.asarray(x, f32)
    pos_emb = np.asarray(pos_emb, f32)
    Wk = np.asarray(Wk, f32)
    W1 = np.asarray(W1, f32)
    b1 = np.asarray(b1, f32)
    ln1_g = np.asarray(ln1_g, f32)
    ln1_b = np.asarray(ln1_b, f32)
    W2 = np.asarray(W2, f32)
    b2 = np.asarray(b2, f32)
    ln2_g = np.asarray(ln2_g, f32)
    ln2_b = np.asarray(ln2_b, f32)
    Wq1 = np.asarray(Wq1, f32)
    bq1 = np.asarray(bq1, f32)
    lnq_g = np.asarray(lnq_g, f32)
    lnq_b = np.asarray(lnq_b, f32)
    Wq2 = np.asarray(Wq2, f32)
    bq2 = np.asarray(bq2, f32)

    trivial = (
        not b1.any() and not b2.any()
        and not ln1_b.any() and not ln2_b.any()
        and np.all(ln1_g == 1.0) and np.all(ln2_g == 1.0)
    )
    if not trivial:
        return _host_reference(x, pos_emb, Wk, W1, b1, ln1_g, ln1_b, W2, b2,
                               ln2_g, ln2_b, Wq1, bq1, lnq_g, lnq_b, Wq2, bq2)

    zL = np.ascontiguousarray(x[:, -1, :])
    attn = _host_query_attn(zL, pos_emb, Wk, Wq1, bq1, lnq_g, lnq_b, Wq2, bq2)

    # ---- select (row, layer) pairs and pack into device slots ----
    # Bins: NCORES slots of each size in RPS.  Repeatedly hand the layer
    # with the most unassigned rows the best-fitting free bin.
    keep = _select_pairs(attn)
    remaining = []                  # [rows_desc_by_attn] per layer
    for l in range(L):
        rows = np.nonzero(keep[:, l])[0]
        if len(rows):
            remaining.append([l, rows[np.argsort(-attn[rows, l],
                                                 kind="stable")]])
    free_bins = {si: NCORES for si in range(NSLOT)}     # slot-idx -> count
    assigned = {si: [] for si in range(NSLOT)}          # slot-idx -> slots
    while any(free_bins.values()) and remaining:
        remaining.sort(key=lambda e: -len(e[1]))
        l, rows = remaining[0]
        r = len(rows)
        # smallest bin that fits everything, else the largest free bin
        pick = None
        for si in sorted(free_bins, key=lambda si: RPS[si]):
            if free_bins[si] and RPS[si] >= r:
                pick = si
                break
        if pick is None:
            pick = max((si for si in free_bins if free_bins[si]),
                       key=lambda si: RPS[si])
        take = min(r, RPS[pick])
        assigned[pick].append((l, rows[:take]))
        free_bins[pick] -= 1
        if take < r:
            remaining[0][1] = rows[take:]
        else:
            remaining.pop(0)
    host_pairs = [(l, rows) for l, rows in remaining]

    # ---- build per-core SPMD inputs ----
    wcache = {}

    def _wslot(l):
        if l not in wcache:
            wcache[l] = (
                np.ascontiguousarray(
                    W1[l].reshape(KD, 128, ND1, 512).transpose(2, 0, 1, 3)
                ).astype(BF16_NP),
                np.ascontiguousarray(
                    W2[l].reshape(KB, 128, ND2, 512).transpose(2, 0, 1, 3)
                ).astype(BF16_NP),
            )
        return wcache[l]

    MBMAX = RPMAX // 128
    in_maps = []
    core_slots = [{} for _ in range(NCORES)]             # slot-idx -> (l, rows)
    for si in range(NSLOT):
        for c, slot in enumerate(assigned[si]):
            core_slots[c][si] = slot
    for c in range(NCORES):
        xts = np.zeros((NSLOT, KD, 128, RPMAX), BF16_NP)
        w1s = np.zeros((NSLOT, ND1, KD, 128, 512), BF16_NP)
        w2s = np.zeros((NSLOT, ND2, KB, 128, 512), BF16_NP)
        ats = np.zeros((NSLOT, MBMAX, 128), f32)
        for si, (l, rows) in core_slots[c].items():
            w1b, w2b = _wslot(l)
            w1s[si] = w1b
            w2s[si] = w2b
            xg = x[rows, l, :].astype(BF16_NP)            # [r, D]
            xts[si, :, :, :len(rows)] = xg.T.reshape(KD, 128, len(rows))
            ats[si].reshape(MBMAX * 128)[:len(rows)] = attn[rows, l]
        in_maps.append({"xt": xts, "w1": w1s, "w2": w2s, "attn": ats})

    nc = _build()
    # Tracing needs the NTFF profile hook; if BASS_TRACE is set in an
    # environment without the hook installed, force-disable tracing so the
    # run doesn't crash on the hook import.
    import os
    we_set_guard = False
    if os.environ.get("BASS_TRACE") and not os.environ.get("BASS_NEVER_TRACE"):
        try:
            from antenv.axon_hooks import get_axon_ntff_profile_hook  # noqa: F401
        except ImportError:
            os.environ["BASS_NEVER_TRACE"] = "1"
            we_set_guard = True
    try:
        res = run_bass_kernel_spmd(nc, in_maps, list(range(NCORES)))
    finally:
        if we_set_guard:
            del os.environ["BASS_NEVER_TRACE"]
    _last_results = res

    acc = zL.copy()
    for c in range(NCORES):
        out_c = res.results[c]["out"].astype(f32)       # [NSLOT,MBMAX,128,D]
        for si, (l, rows) in core_slots[c].items():
            acc[rows] += out_c[si].reshape(MBMAX * 128, D)[:len(rows)]

    # ---- host tail: layers that didn't fit the slot budget (exact fp32) ----
    for l, rows in host_pairs:
        xg = x[rows, l, :]
        h = _ln(_gelu(xg @ W1[l]), 1.0, 0.0)
        v = _ln(h @ W2[l], 1.0, 0.0)
        acc[rows] += attn[rows, l:l + 1] * v

    return acc.astype(f32)
